# revision 5
# baseline (speedup 1.0000x reference)
"""Two-layer GCN + MLP on 8 Trainium2 NeuronCores (v4).

Math: with A-hat = D^-1/2 (A + I) D^-1/2, dinv = deg^-1/2 and self-loops
folded in as explicit (i -> i) edges:
  agg1 = dinv * segsum(y[src]),   y  = dinv * x          (host-precomputed)
  h    = relu(agg1 @ W1 + b1)
  t2   = dinv * (h @ W2)                                  (exchanged, bf16)
  g    = dinv * segsum(t2[src])
  out  = relu(g @ Wm1 + b2@Wm1 + bm1) @ Wm2 + bm2

Sharding: nodes (and edges by destination) split across 8 cores; each core
owns 1250 dst nodes (10 tiles of 128).  The y table (bf16) is built on the
host and replicated to every core's DRAM, so layer-1 gathers start
immediately.  t2 is exchanged in two pieces (tiles 0-5, then 6-9) with
AllGather collectives issued from the SP sequencer, so the Pool-driven
gather stream is never blocked; the first AllGather launches as soon as
tile 5 finishes and overlaps the rest of layer 1.

Layer-2 edges are split per dst tile by which exchange piece holds the
source (A: source loc < 768, B: otherwise), gathering from two separate
tables (tbufA/tbufB) so all A gathers depend only on the first AllGather
and overlap the second.  A-phase partial sums are parked in SBUF (bf16)
and re-injected into the B-phase PSUM group with identity matmuls.

Everything stays transposed (feature-major) between aggregation and the
dense matmuls: per 128-edge chunk, two PE matmuls accumulate
  zT[f_half] += G[:, f_half]^T @ Ind,   Ind[e, d] = (dst_slot[e] == d)
so no PE transposes are needed; biases ride per-partition on the scalar
engine's relu.  All matmul operands are bf16 (full PE rate for 128-wide
moving operands); accumulation stays fp32 in PSUM.
"""
import sys
sys.path.insert(0, "/opt/trn_rl_repo")

import numpy as np

N, D, H = 10000, 256, 512
NC = 8
NSH = N // NC          # 1250 nodes per core
P = 128
NT = 10                # dst tiles per core
NPAD = NT * P          # 1280 padded rows per core
TA = 6                 # tiles in exchange piece A
RA = TA * P            # 768 rows
TB = NT - TA           # 4 tiles in piece B
RB = TB * P            # 512 rows

_cache = {}
_last_res = None
_last_in_maps = None


def _build(key):
    c1, cA, cB = [list(t) for t in key]
    from concourse import bacc, tile, mybir, bass

    f32 = mybir.dt.float32
    bf16 = mybir.dt.bfloat16
    i16 = mybir.dt.int16

    off1 = np.concatenate([[0], np.cumsum(c1)]).astype(int)
    offA = np.concatenate([[0], np.cumsum(cA)]).astype(int)
    offB = np.concatenate([[0], np.cumsum(cB)]).astype(int) + int(offA[-1])
    NCH1 = int(off1[-1])
    NCH2 = int(offB[-1])
    L1 = NCH1 * P
    L2 = NCH2 * P
    gmax = max(max(c1), max(cA), max(cB))
    l1a = c1[0] * 8          # gidx1 columns for tile 0

    nc = bacc.Bacc("TRN2", target_bir_lowering=False, debug=False,
                   enable_asserts=True, num_devices=NC)

    ybuf = nc.dram_tensor("ybuf", [NC * NPAD, D], bf16, kind="ExternalInput").ap()
    gidx1a_d = nc.dram_tensor("gidx1a", [P, l1a], i16, kind="ExternalInput").ap()
    gidx1b_d = nc.dram_tensor("gidx1b", [P, L1 // 16 - l1a], i16,
                              kind="ExternalInput").ap()
    gidx2_d = nc.dram_tensor("gidx2", [P, L2 // 16], i16, kind="ExternalInput").ap()
    dstsel1 = nc.dram_tensor("dstsel1", [P, NCH1], f32, kind="ExternalInput").ap()
    dstsel2 = nc.dram_tensor("dstsel2", [P, NCH2], f32, kind="ExternalInput").ap()
    iota_d = nc.dram_tensor("iota", [P, P], bf16, kind="ExternalInput").ap()
    ident_d = nc.dram_tensor("ident", [P, P], bf16, kind="ExternalInput").ap()
    dinvT_d = nc.dram_tensor("dinvT", [P, NT, P], f32, kind="ExternalInput").ap()
    dinvP_d = nc.dram_tensor("dinvP", [P, NT], f32, kind="ExternalInput").ap()
    W1_d = nc.dram_tensor("W1blk", [P, 2, H], bf16, kind="ExternalInput").ap()
    W2_d = nc.dram_tensor("W2r", [P, 4, D], bf16, kind="ExternalInput").ap()
    Wm1_d = nc.dram_tensor("Wm1blk", [P, 2, H], bf16, kind="ExternalInput").ap()
    Wm2_d = nc.dram_tensor("Wm2r", [P, 4, D], bf16, kind="ExternalInput").ap()
    b1T_d = nc.dram_tensor("b1T", [P, 4], f32, kind="ExternalInput").ap()
    bm1eT_d = nc.dram_tensor("bm1eT", [P, 4], f32, kind="ExternalInput").ap()
    bm2b_d = nc.dram_tensor("bm2b", [P, D], f32, kind="ExternalInput").ap()
    out_d = nc.dram_tensor("out", [NPAD, D], f32, kind="ExternalOutput").ap()

    out_r = out_d.rearrange("(t p) d -> p t d", p=P)

    with tile.TileContext(nc) as tc:
        with tc.tile_pool(name="cst", bufs=1) as cst, \
             tc.tile_pool(name="gpool", bufs=4) as gpool, \
             tc.tile_pool(name="indp", bufs=44) as indp, \
             tc.tile_pool(name="work", bufs=2) as work, \
             tc.tile_pool(name="pz", bufs=2, space="PSUM") as pz, \
             tc.tile_pool(name="ph", bufs=2, space="PSUM") as ph, \
             tc.tile_pool(name="pt", bufs=2, space="PSUM") as pt, \
             tc.tile_pool(name="dram", bufs=1, space="DRAM") as dram:

            # ---------------- constants ----------------
            # tile 0's gather indices live in their own tensor so the first
            # gather doesn't wait for the full index-table load
            gidx1a_t = cst.tile([P, l1a], i16)
            nc.sync.dma_start(out=gidx1a_t[:], in_=gidx1a_d[:])
            gidx1b_t = cst.tile([P, L1 // 16 - l1a], i16)
            nc.sync.dma_start(out=gidx1b_t[:], in_=gidx1b_d[:])
            dst1_t = cst.tile([P, NCH1], f32)
            nc.sync.dma_start(out=dst1_t[:], in_=dstsel1[:])
            iota_t = cst.tile([P, P], bf16)
            nc.sync.dma_start(out=iota_t[:], in_=iota_d[:])
            ident_t = cst.tile([P, P], bf16)
            nc.sync.dma_start(out=ident_t[:], in_=ident_d[:])
            dinvT = cst.tile([P, NT, P], f32)
            nc.sync.dma_start(out=dinvT[:], in_=dinvT_d[:])
            dinvP = cst.tile([P, NT], f32)
            nc.sync.dma_start(out=dinvP[:], in_=dinvP_d[:])
            W1b = cst.tile([P, 2, H], bf16)
            nc.sync.dma_start(out=W1b[:], in_=W1_d[:])
            W2r = cst.tile([P, 4, D], bf16)
            nc.sync.dma_start(out=W2r[:], in_=W2_d[:])
            Wm1b = cst.tile([P, 2, H], bf16)
            nc.sync.dma_start(out=Wm1b[:], in_=Wm1_d[:])
            Wm2r = cst.tile([P, 4, D], bf16)
            nc.sync.dma_start(out=Wm2r[:], in_=Wm2_d[:])
            b1T = cst.tile([P, 4], f32)
            nc.sync.dma_start(out=b1T[:], in_=b1T_d[:])
            bm1eT = cst.tile([P, 4], f32)
            nc.sync.dma_start(out=bm1eT[:], in_=bm1eT_d[:])
            bm2b = cst.tile([P, D], f32)
            nc.sync.dma_start(out=bm2b[:], in_=bm2b_d[:])
            gidx2_t = cst.tile([P, L2 // 16], i16)
            dst2_t = cst.tile([P, NCH2], f32)

            t2A = cst.tile([P, TA, D], bf16, name="t2A")
            t2B = cst.tile([P, TB, D], bf16, name="t2B")
            stageA = dram.tile([RA, D], bf16, name="stageA")
            stageB = dram.tile([RB, D], bf16, name="stageB")
            tbufA = dram.tile([NC * RA, D], bf16, addr_space="Shared",
                              name="tbufA")
            tbufB = dram.tile([NC * RB, D], bf16, addr_space="Shared",
                              name="tbufB")
            zA_all = cst.tile([P, NT, 2, P], bf16, name="zA_all")

            Relu = mybir.ActivationFunctionType.Relu
            Copy = mybir.ActivationFunctionType.Copy
            is_eq = mybir.AluOpType.is_equal
            mult = mybir.AluOpType.mult
            add = mybir.AluOpType.add

            def seg_chunks(psum_zT, g, dst_t, col0, nch, start, stop):
                """zT[f_half] += G[:, f_half]^T @ Ind over nch 128-edge chunks.

                One psum accumulation group spans [start..stop] (single 2KB
                zero region); the two f-halves are disjoint column ranges.
                """
                for i in range(nch):
                    ind = indp.tile([P, P], bf16, name="ind")
                    # Ind builds stay off Pool: the Pool sequencer is in-order
                    # and anything queued between gathers delays the next
                    # gather's descriptor generation.
                    nc.vector.tensor_scalar(out=ind[:], in0=iota_t[:],
                                            scalar1=dst_t[:, col0 + i:col0 + i + 1],
                                            scalar2=None, op0=is_eq)
                    for hh in range(2):
                        nc.tensor.matmul(out=psum_zT[:, hh, :],
                                         lhsT=g[:, i, hh * P:(hh + 1) * P],
                                         rhs=ind[:],
                                         start=(start and i == 0 and hh == 0),
                                         stop=(stop and i == nch - 1 and hh == 1))

            def dense_block(zTs, Wb, biasT, act_out):
                """act_out[:, q, :] = relu(W^T @ zTs + bias), 4 quarters,
                one psum group across all 8 matmuls (shared zero region)."""
                psum_h = ph.tile([P, 4, P], f32, space="PSUM", name="psum_h")
                for q in range(4):
                    for cc2 in range(2):
                        nc.tensor.matmul(out=psum_h[:, q, :],
                                         lhsT=Wb[:, cc2, q * P:(q + 1) * P],
                                         rhs=zTs[:, cc2, :],
                                         start=(q == 0 and cc2 == 0),
                                         stop=(q == 3 and cc2 == 1))
                for q in range(4):
                    nc.scalar.activation(out=act_out[:, q, :],
                                         in_=psum_h[:, q, :], func=Relu,
                                         bias=biasT[:, q:q + 1])

            def out_mm(hT, Wr):
                psum_o = pt.tile([P, D], f32, space="PSUM", name="psum_o",
                                 padded_shape=[P, 512])
                for q in range(4):
                    nc.tensor.matmul(out=psum_o[:], lhsT=hT[:, q, :],
                                     rhs=Wr[:, q, :],
                                     start=(q == 0), stop=(q == 3))
                return psum_o

            def exchange(t2_sb, stage, tbuf_d, eng):
                """Stage piece to DRAM, then AllGather it.  The collective
                must sit on the Pool engine (the only bass-expressible engine
                the BIR verifier accepts for CollectiveCompute on trn2); on
                hardware the sequencer dispatches it to the DMA rings and
                moves on, so dispatching as early as the data allows lets the
                transfer overlap the surrounding gather stream."""
                eng.dma_start(
                    out=stage[:].rearrange("(j p) d -> p j d", p=P),
                    in_=t2_sb[:])
                nc.gpsimd.collective_compute(
                    "AllGather", mybir.AluOpType.bypass,
                    replica_groups=[list(range(NC))],
                    ins=[stage[:].opt()], outs=[tbuf_d[:].opt()],
                )

            def tile_gather(src_ap, gidx_t, col0, nch, nm, split=False):
                """Gather nch 128-edge chunks; split=True uses two DMAs so
                matmuls can start at half-gather."""
                g = gpool.tile([P, nch, D], bf16, name=nm, tag="g",
                               padded_shape=[P, gmax, D])
                parts = ((0, nch // 2), (nch // 2, nch)) \
                    if (split and nch >= 2) else ((0, nch),)
                for a, b in parts:
                    nc.gpsimd.dma_gather(
                        out_ap=g[:, a:b, :], in_ap=src_ap,
                        idxs_ap=gidx_t[:, (col0 + a) * 8:(col0 + b) * 8],
                        num_idxs=(b - a) * P, num_idxs_reg=(b - a) * P,
                        elem_size=D, single_packet=False,
                    )
                return g

            # ---------------- layer 1 ----------------
            def layer1(t, g):
                psum_zT = pz.tile([P, 2, P], f32, space="PSUM", name="psum_zT",
                                  padded_shape=[P, 2, 256])
                seg_chunks(psum_zT, g, dst1_t, int(off1[t]), c1[t],
                           True, True)
                zTs = work.tile([P, 2, P], bf16, name="zTs")
                for hh in range(2):
                    nc.vector.tensor_tensor(out=zTs[:, hh, :],
                                            in0=psum_zT[:, hh, :],
                                            in1=dinvT[:, t], op=mult)
                hT = work.tile([P, 4, P], bf16, name="hT")
                dense_block(zTs, W1b, b1T, hT)
                psum_t2 = out_mm(hT, W2r)
                dst_sb = t2A[:, t, :] if t < TA else t2B[:, t - TA, :]
                nc.scalar.activation(out=dst_sb, in_=psum_t2[:],
                                     func=Copy, scale=dinvP[:, t:t + 1])

            for t in range(NT):
                if t == 0:
                    g = tile_gather(ybuf[:], gidx1a_t, 0, c1[0], "g1")
                else:
                    g = tile_gather(ybuf[:], gidx1b_t,
                                    int(off1[t]) - c1[0], c1[t], "g1")
                layer1(t, g)
                if t == 4:
                    # the layer-2 index/selector tables are needed once the
                    # A phase starts; load them behind the L1 gather stream
                    nc.sync.dma_start(out=gidx2_t[:], in_=gidx2_d[:])
                    nc.sync.dma_start(out=dst2_t[:], in_=dstsel2[:])
                if t == TA - 1:
                    exchange(t2A, stageA, tbufA, nc.sync)
            # second piece: stage from the (otherwise idle) scalar engine so
            # it lands while SP is still held by the first collective
            exchange(t2B, stageB, tbufB, nc.scalar)

            # ---------------- layer 2, phase A ----------------
            def layer2A(t):
                gA = tile_gather(tbufA[:], gidx2_t, int(offA[t]), cA[t], "gA")
                psum_zT = pz.tile([P, 2, P], f32, space="PSUM", name="psum_zA",
                                  tag="psum_zT", padded_shape=[P, 2, 256])
                seg_chunks(psum_zT, gA, dst2_t, int(offA[t]), cA[t],
                           True, True)
                nc.vector.tensor_copy(out=zA_all[:, t], in_=psum_zT[:])

            for t in range(NT):
                layer2A(t)

            # ---------------- layer 2, phase B + MLP ----------------
            def layer2B(t):
                gB = tile_gather(tbufB[:], gidx2_t, int(offB[t]), cB[t], "gB",
                                 split=(t == 9))
                psum_zT = pz.tile([P, 2, P], f32, space="PSUM", name="psum_zB",
                                  tag="psum_zT", padded_shape=[P, 2, 256])
                # re-inject the A-phase partial via identity matmuls
                for hh in range(2):
                    nc.tensor.matmul(out=psum_zT[:, hh, :], lhsT=ident_t[:],
                                     rhs=zA_all[:, t, hh, :],
                                     start=(hh == 0), stop=False)
                seg_chunks(psum_zT, gB, dst2_t, int(offB[t]), cB[t],
                           False, True)
                gTs = work.tile([P, 2, P], bf16, name="gTs")
                for hh in range(2):
                    nc.vector.tensor_tensor(out=gTs[:, hh, :],
                                            in0=psum_zT[:, hh, :],
                                            in1=dinvT[:, t], op=mult)
                o1T = work.tile([P, 4, P], bf16, name="o1T")
                dense_block(gTs, Wm1b, bm1eT, o1T)
                psum_out = out_mm(o1T, Wm2r)
                out_sb = work.tile([P, D], f32, name="out_sb")
                nc.vector.tensor_tensor(out=out_sb[:], in0=psum_out[:],
                                        in1=bm2b[:], op=add)
                nc.sync.dma_start(out=out_r[:, t, :], in_=out_sb[:])

            for t in range(NT):
                layer2B(t)

    nc.finalize()
    return nc


def _wrap16(flat):
    """edge list -> dma_gather int16 index layout [128, len/16]."""
    arr16 = flat.reshape(-1, 16)
    return np.tile(np.ascontiguousarray(arr16.T), (8, 1)).astype(np.int16)


def _prep(edge_index):
    """Host graph preprocessing.

    Adds explicit self-edges (i -> i), shards edges by dst across cores and
    dst tiles, and splits layer-2 edges per tile by which exchange piece
    (A: source loc < RA, B: otherwise) holds the source.  Chunk counts are
    maxed across cores so the SPMD program is shared.
    """
    src = np.asarray(edge_index[0], dtype=np.int64)
    dst = np.asarray(edge_index[1], dtype=np.int64)
    deg = 1 + np.bincount(dst, minlength=N).astype(np.int64)

    all_nodes = np.arange(N, dtype=np.int64)
    src = np.concatenate([src, all_nodes])
    dst = np.concatenate([dst, all_nodes])

    shard = dst // NSH
    loc = dst - shard * NSH
    tile_g = loc // P
    slot = loc % P
    sshard = src // NSH
    soff = src - sshard * NSH
    src1 = sshard * NPAD + soff                       # y-table row
    sB = (soff >= RA).astype(np.int64)
    src2 = np.where(sB == 0, sshard * RA + soff,
                    sshard * RB + (soff - RA))        # row in tbufA / tbufB

    counts1 = np.zeros((NC, NT), np.int64)
    counts2 = np.zeros((NC, NT, 2), np.int64)
    per_core = []
    for k in range(NC):
        sel = shard == k
        t_k = tile_g[sel]
        h_k = sB[sel]
        order1 = np.argsort(t_k, kind="stable")
        order2 = np.lexsort((h_k, t_k))
        e = dict(src1=src1[sel][order1], slot1=slot[sel][order1],
                 src2=src2[sel][order2], slot2=slot[sel][order2])
        per_core.append(e)
        counts1[k] = np.bincount(t_k, minlength=NT)
        for t in range(NT):
            hb = h_k[t_k == t]
            counts2[k, t, 1] = int(hb.sum())
            counts2[k, t, 0] = counts1[k, t] - counts2[k, t, 1]

    # at least one chunk per (tile, piece): an empty psum group is illegal,
    # and an all-pad chunk (idx 0 / selector -1) costs almost nothing
    c1 = tuple(max(1, int(np.ceil(counts1[:, t].max() / P))) for t in range(NT))
    cA = tuple(max(1, int(np.ceil(counts2[:, t, 0].max() / P))) for t in range(NT))
    cB = tuple(max(1, int(np.ceil(counts2[:, t, 1].max() / P))) for t in range(NT))
    key = (c1, cA, cB)

    off1 = np.concatenate([[0], np.cumsum(c1)]).astype(int)
    offA = np.concatenate([[0], np.cumsum(cA)]).astype(int)
    offB = np.concatenate([[0], np.cumsum(cB)]).astype(int) + int(offA[-1])
    L1 = int(off1[-1]) * P
    L2 = int(offB[-1]) * P
    l1a = c1[0] * 8

    arrays = []
    for k in range(NC):
        e = per_core[k]
        idx1 = np.zeros(L1, np.int64)
        sel1 = np.full(L1, -1.0, np.float32)
        idx2 = np.zeros(L2, np.int64)
        sel2 = np.full(L2, -1.0, np.float32)
        pos = 0
        for t in range(NT):
            n = int(counts1[k, t])
            base = int(off1[t]) * P
            idx1[base:base + n] = e["src1"][pos:pos + n]
            sel1[base:base + n] = e["slot1"][pos:pos + n]
            nA = int(counts2[k, t, 0])
            nB = int(counts2[k, t, 1])
            baseA = int(offA[t]) * P
            idx2[baseA:baseA + nA] = e["src2"][pos:pos + nA]
            sel2[baseA:baseA + nA] = e["slot2"][pos:pos + nA]
            baseB = int(offB[t]) * P
            idx2[baseB:baseB + nB] = e["src2"][pos + nA:pos + n]
            sel2[baseB:baseB + nB] = e["slot2"][pos + nA:pos + n]
            pos += n
        g1 = _wrap16(idx1)
        arrays.append(dict(
            gidx1a=np.ascontiguousarray(g1[:, :l1a]),
            gidx1b=np.ascontiguousarray(g1[:, l1a:]),
            gidx2=_wrap16(idx2),
            dstsel1=np.ascontiguousarray(sel1.reshape(-1, P).T),
            dstsel2=np.ascontiguousarray(sel2.reshape(-1, P).T),
        ))
    return deg, arrays, key


def _make_in_maps(x, edge_index, W1, b1, W2, b2, Wm1, bm1, Wm2, bm2):
    import ml_dtypes
    x = np.asarray(x, dtype=np.float32)
    deg, arrays, key = _prep(edge_index)
    dinv = (1.0 / np.sqrt(deg.astype(np.float64))).astype(np.float32)

    # replicated y table (padded, bf16)
    y = x * dinv[:, None]
    yf = np.zeros((NC, NPAD, D), np.float32)
    dv = np.ones((NC, NPAD), np.float32)
    for k in range(NC):
        yf[k, :NSH] = y[k * NSH:(k + 1) * NSH]
        dv[k, :NSH] = dinv[k * NSH:(k + 1) * NSH]
    ybuf = yf.reshape(NC * NPAD, D).astype(ml_dtypes.bfloat16)

    iota = np.tile(np.arange(P, dtype=np.float32), (P, 1)).astype(
        ml_dtypes.bfloat16)
    ident = np.eye(P, dtype=np.float32).astype(ml_dtypes.bfloat16)

    def blk2(W):   # [D, H] -> [128, 2, H] partition = row-within-chunk
        return np.ascontiguousarray(
            np.asarray(W, np.float32).reshape(2, P, H).transpose(1, 0, 2)
        ).astype(ml_dtypes.bfloat16)

    def blk4(W):   # [H, D] -> [128, 4, D]
        return np.ascontiguousarray(
            np.asarray(W, np.float32).reshape(4, P, D).transpose(1, 0, 2)
        ).astype(ml_dtypes.bfloat16)

    W1blk, W2rr = blk2(W1), blk4(W2)
    Wm1blk, Wm2rr = blk2(Wm1), blk4(Wm2)
    b1T = np.ascontiguousarray(np.asarray(b1, np.float32).reshape(4, P).T)
    bm1e = (np.asarray(b2, np.float32) @ np.asarray(Wm1, np.float32)
            + np.asarray(bm1, np.float32))
    bm1eT = np.ascontiguousarray(bm1e.reshape(4, P).T)
    bm2b = np.tile(np.asarray(bm2, np.float32).reshape(1, D), (P, 1))

    in_maps = []
    for k in range(NC):
        dvk = dv[k]                                  # [NPAD]
        row = dvk.reshape(NT, P)                     # [t, j]
        dinvT = np.ascontiguousarray(
            np.broadcast_to(row[None, :, :], (P, NT, P))).astype(np.float32)
        dinvP = np.ascontiguousarray(row.T)          # [p, t]
        in_maps.append(dict(
            ybuf=ybuf, iota=iota, ident=ident, dinvT=dinvT, dinvP=dinvP,
            W1blk=W1blk, W2r=W2rr, Wm1blk=Wm1blk, Wm2r=Wm2rr,
            b1T=b1T, bm1eT=bm1eT, bm2b=bm2b,
            **arrays[k],
        ))
    return in_maps, key


def kernel(x, edge_index, W1, b1, W2, b2, Wm1, bm1, Wm2, bm2):
    from concourse.bass_utils import run_bass_kernel_spmd

    in_maps, key = _make_in_maps(x, edge_index, W1, b1, W2, b2,
                                 Wm1, bm1, Wm2, bm2)
    if key not in _cache:
        _cache[key] = _build(key)
    nc = _cache[key]

    global _last_res, _last_in_maps
    _last_in_maps = in_maps
    res = run_bass_kernel_spmd(nc, in_maps, core_ids=list(range(NC)))
    _last_res = res
    out = np.concatenate(
        [res.results[k]["out"][:NSH] for k in range(NC)], axis=0)
    return out.astype(np.float32)


# revision 6
# speedup vs baseline: 1.3984x; 1.3984x over previous
"""Two-layer GCN + MLP on 8 Trainium2 NeuronCores (v5, ReduceScatter).

Math: with A-hat = D^-1/2 (A + I) D^-1/2, dinv = deg^-1/2 and self-loops
folded in as explicit (i -> i) edges:
  agg1 = dinv * segsum(y[src]),   y  = dinv * x          (host-precomputed)
  h    = relu(agg1 @ W1 + b1)
  t2   = dinv * (h @ W2)
  g    = dinv * segsum(t2[src])
  out  = relu(g @ Wm1 + b2@Wm1 + bm1) @ Wm2 + bm2

Layer 1 is dst-sharded: each core owns 1250 dst nodes (10 tiles of 128)
and gathers from a host-precomputed replicated y table (bf16), so the
gather stream starts immediately.

Layer 2 is src-sharded: each core aggregates partial sums for ALL 80
global dst tiles using only the edges whose SOURCE it owns -- its t2 rows
never leave the core before aggregation, so no collective is needed until
the very end.  The feature-major partial blocks are staged to DRAM
(owner-major order) and a single ReduceScatter(add) hands every core the
fully-summed rows for its own 10 tiles, which then run through the MLP.
The ReduceScatter moves only 1/8 of the table per core -- the collective
on the critical path is ~5x smaller than an AllGather of t2.

Everything stays transposed (feature-major) between aggregation and the
dense matmuls: per 128-edge chunk, two PE matmuls accumulate
  zT[f_half] += G[:, f_half]^T @ Ind,   Ind[e, d] = (dst_slot[e] == d)
so no PE transposes are needed; biases ride per-partition on the scalar
engine's relu.  All matmul operands are bf16 (full PE rate for 128-wide
moving operands); accumulation stays fp32 in PSUM.
"""
import sys
sys.path.insert(0, "/opt/trn_rl_repo")

import numpy as np

N, D, H = 10000, 256, 512
NC = 8
NSH = N // NC          # 1250 nodes per core
P = 128
NT = 10                # dst tiles per core
NPAD = NT * P          # 1280 padded rows per core
GT = NC * NT           # 80 global dst tiles

_cache = {}
_last_res = None
_last_in_maps = None


def _build(key):
    c1, c2 = [list(t) for t in key]
    from concourse import bacc, tile, mybir

    f32 = mybir.dt.float32
    bf16 = mybir.dt.bfloat16
    i16 = mybir.dt.int16

    off1 = np.concatenate([[0], np.cumsum(c1)]).astype(int)
    off2 = np.concatenate([[0], np.cumsum(c2)]).astype(int)
    NCH1 = int(off1[-1])
    NCH2 = int(off2[-1])
    L1 = NCH1 * P
    L2 = NCH2 * P
    gmax = max(max(c1), max(c2))
    l1a = c1[0] * 8          # gidx1 columns for tile 0

    nc = bacc.Bacc("TRN2", target_bir_lowering=False, debug=False,
                   enable_asserts=True, num_devices=NC)

    ybuf = nc.dram_tensor("ybuf", [NC * NPAD, D], bf16, kind="ExternalInput").ap()
    gidx1a_d = nc.dram_tensor("gidx1a", [P, l1a], i16, kind="ExternalInput").ap()
    gidx1b_d = nc.dram_tensor("gidx1b", [P, L1 // 16 - l1a], i16,
                              kind="ExternalInput").ap()
    gidx2_d = nc.dram_tensor("gidx2", [P, L2 // 16], i16, kind="ExternalInput").ap()
    dstsel1 = nc.dram_tensor("dstsel1", [P, NCH1], f32, kind="ExternalInput").ap()
    dstsel2 = nc.dram_tensor("dstsel2", [P, NCH2], f32, kind="ExternalInput").ap()
    iota_d = nc.dram_tensor("iota", [P, P], bf16, kind="ExternalInput").ap()
    dinvT_d = nc.dram_tensor("dinvT", [P, NT, P], f32, kind="ExternalInput").ap()
    dinvP_d = nc.dram_tensor("dinvP", [P, NT], f32, kind="ExternalInput").ap()
    W1_d = nc.dram_tensor("W1blk", [P, 2, H], bf16, kind="ExternalInput").ap()
    W2_d = nc.dram_tensor("W2r", [P, 4, D], bf16, kind="ExternalInput").ap()
    Wm1_d = nc.dram_tensor("Wm1blk", [P, 2, H], bf16, kind="ExternalInput").ap()
    Wm2_d = nc.dram_tensor("Wm2r", [P, 4, D], bf16, kind="ExternalInput").ap()
    b1T_d = nc.dram_tensor("b1T", [P, 4], f32, kind="ExternalInput").ap()
    bm1eT_d = nc.dram_tensor("bm1eT", [P, 4], f32, kind="ExternalInput").ap()
    bm2b_d = nc.dram_tensor("bm2b", [P, D], f32, kind="ExternalInput").ap()
    out_d = nc.dram_tensor("out", [NPAD, D], f32, kind="ExternalOutput").ap()

    out_r = out_d.rearrange("(t p) d -> p t d", p=P)

    with tile.TileContext(nc) as tc:
        with tc.tile_pool(name="cst", bufs=1) as cst, \
             tc.tile_pool(name="gpool", bufs=4) as gpool, \
             tc.tile_pool(name="indp", bufs=44) as indp, \
             tc.tile_pool(name="work", bufs=2) as work, \
             tc.tile_pool(name="zp", bufs=3) as zp, \
             tc.tile_pool(name="pz", bufs=2, space="PSUM") as pz, \
             tc.tile_pool(name="ph", bufs=2, space="PSUM") as ph, \
             tc.tile_pool(name="pt", bufs=2, space="PSUM") as pt, \
             tc.tile_pool(name="dram", bufs=1, space="DRAM") as dram:

            # ---------------- constants ----------------
            gidx1a_t = cst.tile([P, l1a], i16)
            nc.sync.dma_start(out=gidx1a_t[:], in_=gidx1a_d[:])
            gidx1b_t = cst.tile([P, L1 // 16 - l1a], i16)
            nc.sync.dma_start(out=gidx1b_t[:], in_=gidx1b_d[:])
            dst1_t = cst.tile([P, NCH1], f32)
            nc.sync.dma_start(out=dst1_t[:], in_=dstsel1[:])
            iota_t = cst.tile([P, P], bf16)
            nc.sync.dma_start(out=iota_t[:], in_=iota_d[:])
            dinvT = cst.tile([P, NT, P], f32)
            nc.sync.dma_start(out=dinvT[:], in_=dinvT_d[:])
            dinvP = cst.tile([P, NT], f32)
            nc.sync.dma_start(out=dinvP[:], in_=dinvP_d[:])
            W1b = cst.tile([P, 2, H], bf16)
            nc.sync.dma_start(out=W1b[:], in_=W1_d[:])
            W2r = cst.tile([P, 4, D], bf16)
            nc.sync.dma_start(out=W2r[:], in_=W2_d[:])
            Wm1b = cst.tile([P, 2, H], bf16)
            nc.sync.dma_start(out=Wm1b[:], in_=Wm1_d[:])
            Wm2r = cst.tile([P, 4, D], bf16)
            nc.sync.dma_start(out=Wm2r[:], in_=Wm2_d[:])
            b1T = cst.tile([P, 4], f32)
            nc.sync.dma_start(out=b1T[:], in_=b1T_d[:])
            bm1eT = cst.tile([P, 4], f32)
            nc.sync.dma_start(out=bm1eT[:], in_=bm1eT_d[:])
            bm2b = cst.tile([P, D], f32)
            nc.sync.dma_start(out=bm2b[:], in_=bm2b_d[:])
            gidx2_t = cst.tile([P, L2 // 16], i16)
            dst2_t = cst.tile([P, NCH2], f32)

            t2stage = dram.tile([NPAD, D], bf16, name="t2stage")
            t2stage_r = t2stage[:].rearrange("(t p) d -> p t d", p=P)
            partial_d = dram.tile([GT, P, 2 * P], bf16, name="partial_d")
            rsout = dram.tile([NT, P, 2 * P], bf16, name="rsout")

            Relu = mybir.ActivationFunctionType.Relu
            Copy = mybir.ActivationFunctionType.Copy
            is_eq = mybir.AluOpType.is_equal
            mult = mybir.AluOpType.mult
            add = mybir.AluOpType.add

            def seg_chunks(psum_zT, g, dst_t, col0, nch):
                """zT[f_half] += G[:, f_half]^T @ Ind over nch 128-edge
                chunks; one psum accumulation group (single 2KB region)."""
                for i in range(nch):
                    ind = indp.tile([P, P], bf16, name="ind")
                    # Ind builds stay off Pool: the Pool sequencer is in-order
                    # and anything queued between gathers delays the next
                    # gather's descriptor generation.
                    nc.vector.tensor_scalar(out=ind[:], in0=iota_t[:],
                                            scalar1=dst_t[:, col0 + i:col0 + i + 1],
                                            scalar2=None, op0=is_eq)
                    for hh in range(2):
                        nc.tensor.matmul(out=psum_zT[:, hh, :],
                                         lhsT=g[:, i, hh * P:(hh + 1) * P],
                                         rhs=ind[:],
                                         start=(i == 0 and hh == 0),
                                         stop=(i == nch - 1 and hh == 1))

            def dense_block(zTs, Wb, biasT, act_out):
                """act_out[:, q, :] = relu(W^T @ zTs + bias), 4 quarters,
                one psum group across all 8 matmuls (shared zero region)."""
                psum_h = ph.tile([P, 4, P], f32, space="PSUM", name="psum_h")
                for q in range(4):
                    for cc2 in range(2):
                        nc.tensor.matmul(out=psum_h[:, q, :],
                                         lhsT=Wb[:, cc2, q * P:(q + 1) * P],
                                         rhs=zTs[:, cc2, :],
                                         start=(q == 0 and cc2 == 0),
                                         stop=(q == 3 and cc2 == 1))
                for q in range(4):
                    nc.scalar.activation(out=act_out[:, q, :],
                                         in_=psum_h[:, q, :], func=Relu,
                                         bias=biasT[:, q:q + 1])

            def out_mm(hT, Wr):
                psum_o = pt.tile([P, D], f32, space="PSUM", name="psum_o",
                                 padded_shape=[P, 512])
                for q in range(4):
                    nc.tensor.matmul(out=psum_o[:], lhsT=hT[:, q, :],
                                     rhs=Wr[:, q, :],
                                     start=(q == 0), stop=(q == 3))
                return psum_o

            def tile_gather(src_ap, gidx_t, col0, nch, nm, split=False):
                g = gpool.tile([P, nch, D], bf16, name=nm, tag="g",
                               padded_shape=[P, gmax, D])
                parts = ((0, nch // 2), (nch // 2, nch)) \
                    if (split and nch >= 2) else ((0, nch),)
                for a, b in parts:
                    nc.gpsimd.dma_gather(
                        out_ap=g[:, a:b, :], in_ap=src_ap,
                        idxs_ap=gidx_t[:, (col0 + a) * 8:(col0 + b) * 8],
                        num_idxs=(b - a) * P, num_idxs_reg=(b - a) * P,
                        elem_size=D, single_packet=False,
                    )
                return g

            # ---------------- layer 1 (dst-sharded) ----------------
            def layer1(t, g):
                psum_zT = pz.tile([P, 2, P], f32, space="PSUM", name="psum_zT",
                                  padded_shape=[P, 2, 256])
                seg_chunks(psum_zT, g, dst1_t, int(off1[t]), c1[t])
                zTs = work.tile([P, 2, P], bf16, name="zTs")
                for hh in range(2):
                    nc.vector.tensor_tensor(out=zTs[:, hh, :],
                                            in0=psum_zT[:, hh, :],
                                            in1=dinvT[:, t], op=mult)
                hT = work.tile([P, 4, P], bf16, name="hT")
                dense_block(zTs, W1b, b1T, hT)
                psum_t2 = out_mm(hT, W2r)
                t2g = work.tile([P, D], bf16, name="t2g")
                nc.scalar.activation(out=t2g[:], in_=psum_t2[:],
                                     func=Copy, scale=dinvP[:, t:t + 1])
                nc.sync.dma_start(out=t2stage_r[:, t, :], in_=t2g[:])

            for t in range(NT):
                if t == 0:
                    g = tile_gather(ybuf[:], gidx1a_t, 0, c1[0], "g1")
                else:
                    g = tile_gather(ybuf[:], gidx1b_t,
                                    int(off1[t]) - c1[0], c1[t], "g1",
                                    split=(t == 9))
                layer1(t, g)
                if t == 4:
                    # the layer-2 tables are needed right after layer 1;
                    # load them behind the L1 gather stream
                    nc.sync.dma_start(out=gidx2_t[:], in_=gidx2_d[:])
                    nc.sync.dma_start(out=dst2_t[:], in_=dstsel2[:])

            # ------- layer 2: src-sharded partials over 80 global tiles ----
            for gt in range(GT):
                g2 = tile_gather(t2stage[:], gidx2_t, int(off2[gt]), c2[gt],
                                 "g2")
                psum_zT = pz.tile([P, 2, P], f32, space="PSUM", name="psum_zP",
                                  tag="psum_zT", padded_shape=[P, 2, 256])
                seg_chunks(psum_zT, g2, dst2_t, int(off2[gt]), c2[gt])
                zP = zp.tile([P, 2 * P], bf16, name="zP")
                nc.vector.tensor_copy(out=zP[:], in_=psum_zT[:])
                nc.sync.dma_start(out=partial_d[gt], in_=zP[:])

            # one ReduceScatter(add): each core receives the summed
            # feature-major blocks for its own 10 tiles
            nc.gpsimd.collective_compute(
                "ReduceScatter", add,
                replica_groups=[list(range(NC))],
                ins=[partial_d[:].opt()], outs=[rsout[:].opt()],
            )

            # ---------------- MLP on the summed rows ----------------
            for t in range(NT):
                z2 = zp.tile([P, 2, P], bf16, name="z2")
                nc.sync.dma_start(
                    out=z2[:], in_=rsout[t].rearrange("f (h d) -> f h d", h=2))
                gTs = work.tile([P, 2, P], bf16, name="gTs")
                for hh in range(2):
                    nc.vector.tensor_tensor(out=gTs[:, hh, :],
                                            in0=z2[:, hh, :],
                                            in1=dinvT[:, t], op=mult)
                o1T = work.tile([P, 4, P], bf16, name="o1T")
                dense_block(gTs, Wm1b, bm1eT, o1T)
                psum_out = out_mm(o1T, Wm2r)
                out_sb = work.tile([P, D], f32, name="out_sb")
                nc.vector.tensor_tensor(out=out_sb[:], in0=psum_out[:],
                                        in1=bm2b[:], op=add)
                nc.sync.dma_start(out=out_r[:, t, :], in_=out_sb[:])

    nc.finalize()
    return nc


def _wrap16(flat):
    """edge list -> dma_gather int16 index layout [128, len/16]."""
    arr16 = flat.reshape(-1, 16)
    return np.tile(np.ascontiguousarray(arr16.T), (8, 1)).astype(np.int16)


def _prep(edge_index):
    """Host graph preprocessing.

    Layer 1: edges (plus explicit self-edges) sharded by dst core and dst
    tile, indices into the replicated y table.
    Layer 2: the SAME edges sharded by src core, grouped by global dst
    tile, indices into the local t2 staging table (src local index).
    Chunk counts are maxed across cores so the SPMD program is shared.
    """
    src = np.asarray(edge_index[0], dtype=np.int64)
    dst = np.asarray(edge_index[1], dtype=np.int64)
    deg = 1 + np.bincount(dst, minlength=N).astype(np.int64)

    all_nodes = np.arange(N, dtype=np.int64)
    src = np.concatenate([src, all_nodes])
    dst = np.concatenate([dst, all_nodes])

    shard = dst // NSH
    loc = dst - shard * NSH
    tile_g = loc // P
    slot = loc % P
    gtile = shard * NT + tile_g                       # global dst tile
    sshard = src // NSH
    soff = src - sshard * NSH
    src1 = sshard * NPAD + soff                       # y-table row
    src2 = soff                                       # local t2stage row

    counts1 = np.zeros((NC, NT), np.int64)
    counts2 = np.zeros((NC, GT), np.int64)
    per_core = []
    for k in range(NC):
        sel1 = shard == k
        t_k = tile_g[sel1]
        order1 = np.argsort(t_k, kind="stable")
        sel2 = sshard == k
        gt_k = gtile[sel2]
        order2 = np.argsort(gt_k, kind="stable")
        per_core.append(dict(
            src1=src1[sel1][order1], slot1=slot[sel1][order1],
            src2=src2[sel2][order2], slot2=slot[sel2][order2],
            gt2=gt_k[order2]))
        counts1[k] = np.bincount(t_k, minlength=NT)
        counts2[k] = np.bincount(gt_k, minlength=GT)

    c1 = tuple(max(1, int(np.ceil(counts1[:, t].max() / P)))
               for t in range(NT))
    c2 = tuple(max(1, int(np.ceil(counts2[:, g].max() / P)))
               for g in range(GT))
    key = (c1, c2)

    off1 = np.concatenate([[0], np.cumsum(c1)]).astype(int)
    off2 = np.concatenate([[0], np.cumsum(c2)]).astype(int)
    L1 = int(off1[-1]) * P
    L2 = int(off2[-1]) * P
    l1a = c1[0] * 8

    arrays = []
    for k in range(NC):
        e = per_core[k]
        idx1 = np.zeros(L1, np.int64)
        sel1v = np.full(L1, -1.0, np.float32)
        idx2 = np.zeros(L2, np.int64)
        sel2v = np.full(L2, -1.0, np.float32)
        pos = 0
        for t in range(NT):
            n = int(counts1[k, t])
            base = int(off1[t]) * P
            idx1[base:base + n] = e["src1"][pos:pos + n]
            sel1v[base:base + n] = e["slot1"][pos:pos + n]
            pos += n
        pos = 0
        for gt in range(GT):
            n = int(counts2[k, gt])
            base = int(off2[gt]) * P
            idx2[base:base + n] = e["src2"][pos:pos + n]
            sel2v[base:base + n] = e["slot2"][pos:pos + n]
            pos += n
        g1 = _wrap16(idx1)
        arrays.append(dict(
            gidx1a=np.ascontiguousarray(g1[:, :l1a]),
            gidx1b=np.ascontiguousarray(g1[:, l1a:]),
            gidx2=_wrap16(idx2),
            dstsel1=np.ascontiguousarray(sel1v.reshape(-1, P).T),
            dstsel2=np.ascontiguousarray(sel2v.reshape(-1, P).T),
        ))
    return deg, arrays, key


def _make_in_maps(x, edge_index, W1, b1, W2, b2, Wm1, bm1, Wm2, bm2):
    import ml_dtypes
    x = np.asarray(x, dtype=np.float32)
    deg, arrays, key = _prep(edge_index)
    dinv = (1.0 / np.sqrt(deg.astype(np.float64))).astype(np.float32)

    # replicated y table (padded, bf16)
    y = x * dinv[:, None]
    yf = np.zeros((NC, NPAD, D), np.float32)
    dv = np.ones((NC, NPAD), np.float32)
    for k in range(NC):
        yf[k, :NSH] = y[k * NSH:(k + 1) * NSH]
        dv[k, :NSH] = dinv[k * NSH:(k + 1) * NSH]
    ybuf = yf.reshape(NC * NPAD, D).astype(ml_dtypes.bfloat16)

    iota = np.tile(np.arange(P, dtype=np.float32), (P, 1)).astype(
        ml_dtypes.bfloat16)

    def blk2(W):
        return np.ascontiguousarray(
            np.asarray(W, np.float32).reshape(2, P, H).transpose(1, 0, 2)
        ).astype(ml_dtypes.bfloat16)

    def blk4(W):
        return np.ascontiguousarray(
            np.asarray(W, np.float32).reshape(4, P, D).transpose(1, 0, 2)
        ).astype(ml_dtypes.bfloat16)

    W1blk, W2rr = blk2(W1), blk4(W2)
    Wm1blk, Wm2rr = blk2(Wm1), blk4(Wm2)
    b1T = np.ascontiguousarray(np.asarray(b1, np.float32).reshape(4, P).T)
    bm1e = (np.asarray(b2, np.float32) @ np.asarray(Wm1, np.float32)
            + np.asarray(bm1, np.float32))
    bm1eT = np.ascontiguousarray(bm1e.reshape(4, P).T)
    bm2b = np.tile(np.asarray(bm2, np.float32).reshape(1, D), (P, 1))

    in_maps = []
    for k in range(NC):
        dvk = dv[k]
        row = dvk.reshape(NT, P)
        dinvT = np.ascontiguousarray(
            np.broadcast_to(row[None, :, :], (P, NT, P))).astype(np.float32)
        dinvP = np.ascontiguousarray(row.T)
        in_maps.append(dict(
            ybuf=ybuf, iota=iota, dinvT=dinvT, dinvP=dinvP,
            W1blk=W1blk, W2r=W2rr, Wm1blk=Wm1blk, Wm2r=Wm2rr,
            b1T=b1T, bm1eT=bm1eT, bm2b=bm2b,
            **arrays[k],
        ))
    return in_maps, key


def kernel(x, edge_index, W1, b1, W2, b2, Wm1, bm1, Wm2, bm2):
    from concourse.bass_utils import run_bass_kernel_spmd

    in_maps, key = _make_in_maps(x, edge_index, W1, b1, W2, b2,
                                 Wm1, bm1, Wm2, bm2)
    if key not in _cache:
        _cache[key] = _build(key)
    nc = _cache[key]

    global _last_res, _last_in_maps
    _last_in_maps = in_maps
    res = run_bass_kernel_spmd(nc, in_maps, core_ids=list(range(NC)))
    _last_res = res
    out = np.concatenate(
        [res.results[k]["out"][:NSH] for k in range(NC)], axis=0)
    return out.astype(np.float32)
